# revision 28
# baseline (speedup 1.0000x reference)
"""Boids simulation step on 8 TRN2 NeuronCores (Bass/Tile).

Problem: B=4 independent sims of N=2048 boids in 2D. One step:
  - N x N pairwise: mask = (dist2 < R2) & (dist2 > 0); w = mask / (dist2 + EPS)
  - alignment / cohesion: (mask @ vel, mask @ pos, row-count) * 1/(cnt+eps)
  - separation: (w @ pos) - pos * rowsum(w)
  - steer = limit(align + coh + 1.5*sep, 0.5); vel' = limit(vel + steer, 0.3)
  - pos' = (pos + vel') % 1.0   (this jax lowers % as x - round(x), so the
    wrap is exactly x - (x >= 0.5) for x in (-0.3, 1.3))

Sharding: 8 cores = 4 batches x 2 halves. The host sorts each batch by x;
each core owns 1024 contiguous sorted query rows and gets the full 2048
keys ROLLED so its queries sit at key rows [0, 1024). The roll makes both
the i==j diagonal position and the x-band structure core-independent, so
one SPMD kernel serves all cores.

x-band culling: boids interact only within x-distance 0.1, so a 128-key
chunk only needs the ~(0.1 + chunk width) * N sorted-index band of query
columns (BAND_D slack, verified exactly on the host per call; falls back
to a full-band kernel variant if the input is too non-uniform). This
skips ~56% of the N^2 work including 4 of 16 chunks entirely.

Per core, for each of the 12 active key chunks (128 keys x W queries):
  dx2 = Square(ximat + (-xj))   [ScalarE: fused sub+square, bitwise == ref]
  dy2 = Square(yimat + (-yj))   [ScalarE]
  dist2 = dx2 + dy2             [VectorE, bitwise == ref]
  mask = dist2 < R2             [VectorE, exact 0/1; diag chunk zeroed]
  d2e = dist2 + EPS             [ScalarE Identity+bias]
  r = ~1/d2e                    [VectorE custom op, ~3e-6 rel]
  w = mask * r                  [VectorE]
  PE (V5 stationary so weight loads are ~free, tiles moving):
    psA(5,cols) += V5.T @ mask; psB(3,cols) += P3.T @ w  (PSUM accum)
The (8, 1024) sums are PE-transposed to (128, chunk, 8) planar form and a
paired-xy epilogue mirrors the reference ops bit-for-bit where it matters.
All DRAM I/O uses partition-major (128, chunk, 2) layouts so DMAs are
contiguous (the host does the cheap reshapes + unsort).
"""
import os
import numpy as np
from contextlib import ExitStack

B = 4
N = 2048
NQ = 1024          # query rows per core
NCORES = 8
P = 128
JT = N // P        # 16 key chunks
G = NQ // P        # 8 query sub-tiles
R2 = np.float32(0.1 ** 2)
EPS = np.float32(1e-6)
SW = np.float32(1.5)
MAX_SPEED = np.float32(0.3)
MAX_FORCE = np.float32(0.5)

_NC = {}

GP_TT = os.environ.get("BOIDS_GP_TT", "0") == "1"   # dist2 add on GPSIMD
D2E_ACT = os.environ.get("BOIDS_D2E", "act") == "act"
F32R = os.environ.get("BOIDS_F32R", "0") == "1"
BCAST_MM = os.environ.get("BOIDS_BCAST", "dma") == "mm"
SQRT_WARM = os.environ.get("BOIDS_SQRTWARM", "1") == "1"
# x-band culling: boids sorted by x only interact within sorted-index
# distance ~ (0.1 + chunk width) * N. D = slack in sorted indices.
BAND_D = int(os.environ.get("BOIDS_BAND_D", "250"))
FORCE_FULL = os.environ.get("BOIDS_FULL", "0") == "1"


def _compute_bands(full):
    """Per key-chunk query-column band [b0, b1) (128-aligned) or None."""
    if full:
        return [(0, NQ)] * JT
    bands = []
    for jt in range(JT):
        lo, hi = jt * P - BAND_D, jt * P + P + BAND_D
        iv = None
        for off in (-N, 0, N):
            a, b_ = max(lo + off, 0), min(hi + off, NQ)
            if a < b_:
                assert iv is None  # arc < 1024 cannot clip to two intervals
                iv = (a, b_)
        if iv is None:
            bands.append(None)
        else:
            bands.append(((iv[0] // P) * P, min(NQ, ((iv[1] + P - 1) // P) * P)))
    return bands


def _build_nc(full=False):
    import concourse.tile as tile
    from concourse import mybir, bacc

    f32 = mybir.dt.float32
    u32 = mybir.dt.uint32
    Alu = mybir.AluOpType
    Act = mybir.ActivationFunctionType

    bands = _compute_bands(full)
    nc = bacc.Bacc("TRN2", target_bir_lowering=False, debug=False)

    # all I/O in partition-major (128, chunk, xy) layout so DMAs are
    # contiguous per partition; the host does the cheap reshapes
    qpos = nc.dram_tensor("qpos", [P, G, 2], f32, kind="ExternalInput")
    qvel = nc.dram_tensor("qvel", [P, G, 2], f32, kind="ExternalInput")
    qposT = nc.dram_tensor("qposT", [2, NQ], f32, kind="ExternalInput")
    kpos = nc.dram_tensor("kpos", [P, JT, 2], f32, kind="ExternalInput")
    kvel = nc.dram_tensor("kvel", [P, JT, 2], f32, kind="ExternalInput")
    opos = nc.dram_tensor("opos", [P, G, 2], f32, kind="ExternalOutput")
    ovel = nc.dram_tensor("ovel", [P, G, 2], f32, kind="ExternalOutput")

    with tile.TileContext(nc) as tc, ExitStack() as ctx:
        const = ctx.enter_context(tc.tile_pool(name="const", bufs=1))
        WORK_BUFS = int(os.environ.get("BOIDS_BUFS", "3"))
        work = ctx.enter_context(tc.tile_pool(name="work", bufs=WORK_BUFS))
        ep = ctx.enter_context(tc.tile_pool(name="ep", bufs=1))
        psum = ctx.enter_context(tc.tile_pool(name="psum", bufs=1, space="PSUM"))
        psumt = ctx.enter_context(tc.tile_pool(name="psumt", bufs=2, space="PSUM"))
        psumb = ctx.enter_context(tc.tile_pool(name="psumb", bufs=2, space="PSUM"))

        # ---- setup -----------------------------------------------------
        # Query coords broadcast across partitions (partition-stride-0 DMA),
        # split in halves across queues; first so distance tiles start ASAP.
        ximat = const.tile([P, NQ], f32)
        yimat = const.tile([P, NQ], f32)
        if BCAST_MM:
            qTx = const.tile([1, NQ], f32)
            qTy = const.tile([1, NQ], f32)
            nc.sync.dma_start(out=qTx, in_=qposT[0:1, :])
            nc.sync.dma_start(out=qTy, in_=qposT[1:2, :])
            ones1 = const.tile([1, P], f32)
            nc.vector.memset(ones1, 1.0)
            for half in range(2):
                cs = slice(half * 512, (half + 1) * 512)
                for c, dstm in ((0, ximat), (1, yimat)):
                    bc = psumb.tile([P, 512], f32, tag="bc", name=f"bc{c}{half}")
                    src = qTx if c == 0 else qTy
                    nc.tensor.matmul(bc, lhsT=ones1, rhs=src[0:1, cs],
                                     start=True, stop=True)
                    (nc.vector.tensor_copy if c == 0 else nc.scalar.copy)(
                        dstm[:, cs], bc)
        else:
            nc.sync.dma_start(out=ximat[:, 0:512],
                              in_=qposT[0:1, 0:512].to_broadcast((P, 512)))
            nc.gpsimd.dma_start(out=yimat[:, 0:512],
                                in_=qposT[1:2, 0:512].to_broadcast((P, 512)))
            nc.scalar.dma_start(out=ximat[:, 512:NQ],
                                in_=qposT[0:1, 512:NQ].to_broadcast((P, 512)))
            nc.sync.dma_start(out=yimat[:, 512:NQ],
                              in_=qposT[1:2, 512:NQ].to_broadcast((P, 512)))

        vmat = const.tile([P, JT, 5], f32)
        nc.gpsimd.dma_start(out=vmat[:, :, 0:2], in_=kvel[:, :, :])
        nc.scalar.dma_start(out=vmat[:, :, 2:4], in_=kpos[:, :, :])
        nc.vector.memset(vmat[:, :, 4], 1.0)

        nkx = const.tile([P, JT], f32)
        nky = const.tile([P, JT], f32)
        nc.vector.tensor_scalar_mul(nkx, vmat[:, :, 2], -1.0)
        nc.vector.tensor_scalar_mul(nky, vmat[:, :, 3], -1.0)

        qp = const.tile([P, G, 2], f32)
        qv = const.tile([P, G, 2], f32)
        nc.sync.dma_start(out=qp, in_=qpos[:, :, :])
        nc.gpsimd.dma_start(out=qv, in_=qvel[:, :, :])

        ones_g = const.tile([P, G], f32)
        nc.vector.memset(ones_g, 1.0)

        # anti-identity (1 everywhere, 0 on diagonal) for the i==j zeroing
        antiI = const.tile([P, P], f32)
        nc.vector.memset(antiI, 1.0)
        nc.gpsimd.affine_select(
            antiI, antiI, pattern=[[1, P]],
            compare_op=Alu.not_equal, fill=0.0,
            base=0, channel_multiplier=-1)

        # 8x8 identity for PE-transpose of the (8, 1024) sums
        ident8 = const.tile([8, 8], f32)
        nc.vector.memset(ident8, 1.0)
        nc.gpsimd.affine_select(
            ident8, ident8, pattern=[[1, 8]],
            compare_op=Alu.is_equal, fill=0.0,
            base=0, channel_multiplier=-1)

        S = const.tile([P, G, 8], f32)  # per-query sums, planar
        mmdt = mybir.dt.float32r if F32R else f32

        eps_col = const.tile([P, 1], f32)
        nc.vector.memset(eps_col, float(EPS))
        if SQRT_WARM:
            sqrt_warm = const.tile([P, 1], f32)
            nc.scalar.activation(sqrt_warm, eps_col, Act.Sqrt)

        zeros512 = const.tile([P, 512], f32)
        nc.gpsimd.memset(zeros512, 0.0)

        # ---- main N^2 loop (16 x (128 keys x banded queries)) ---------
        psA = [psum.tile([5, 512], f32, tag=f"psA{i}", name=f"psA{i}")
               for i in range(2)]
        psB = [psum.tile([3, 512], f32, tag=f"psB{i}", name=f"psB{i}")
               for i in range(2)]
        # pre-clear accumulators (bands leave per-element first-writers
        # mixed); lhsT values are irrelevant since rhs == 0
        for pt, lh in ((psA[0], antiI[:, 0:5]), (psA[1], antiI[:, 0:5]),
                       (psB[0], antiI[:, 0:3]), (psB[1], antiI[:, 0:3])):
            nc.tensor.matmul(pt, lhsT=lh, rhs=zeros512, start=True,
                             stop=False, skip_group_check=True)
        # last chunk writing each 512-column half (for the stop flag)
        lastw = [None, None]
        for jt in range(JT):
            if bands[jt] is None:
                continue
            b0, b1 = bands[jt]
            for half in range(2):
                if b0 < (half + 1) * 512 and b1 > half * 512:
                    lastw[half] = jt

        for jt in range(JT):
            if bands[jt] is None:
                continue
            b0, b1 = bands[jt]
            W = b1 - b0
            dx2 = work.tile([P, W], f32, tag="dx2", name="dx2")
            nc.scalar.activation(dx2, ximat[:, b0:b1], Act.Square,
                                 bias=nkx[:, jt:jt + 1], scale=1.0)
            dy2 = work.tile([P, W], f32, tag="dy2", name="dy2")
            nc.scalar.activation(dy2, yimat[:, b0:b1], Act.Square,
                                 bias=nky[:, jt:jt + 1], scale=1.0)
            dist2 = work.tile([P, W], f32, tag="dist2", name="dist2")
            (nc.gpsimd if GP_TT else nc.vector).tensor_tensor(
                dist2, dx2, dy2, op=Alu.add)

            mask = work.tile([P, W], f32, tag="mask", name="mask")
            nc.vector.tensor_scalar(mask, dist2, float(R2), None,
                                    op0=Alu.is_lt)
            if jt < G:  # this chunk contains the i==j diagonal
                off = jt * P - b0
                assert 0 <= off and off + P <= W
                sub = mask[:, off:off + P]
                nc.vector.tensor_tensor(sub, sub, antiI, op=Alu.mult)

            d2e = work.tile([P, W], f32, tag="d2e", name="d2e")
            if D2E_ACT:
                nc.scalar.activation(d2e, dist2, Act.Identity,
                                     bias=eps_col[:, 0:1], scale=1.0)
            else:
                nc.vector.tensor_scalar(d2e, dist2, float(EPS), None,
                                        op0=Alu.add)
            r = work.tile([P, W], f32, tag="r", name="r")
            nc.vector.reciprocal_approx_fast(out=r, in_=d2e)
            w = work.tile([P, W], f32, tag="w", name="w")
            nc.vector.tensor_tensor(w, mask, r, op=Alu.mult)

            for half in range(2):
                c0, c1 = max(b0, half * 512), min(b1, (half + 1) * 512)
                if c0 >= c1:
                    continue
                stop = (lastw[half] == jt)
                nc.tensor.matmul(psA[half][:, c0 - half * 512:c1 - half * 512],
                                 lhsT=vmat[:, jt, :].bitcast(mmdt),
                                 rhs=mask[:, c0 - b0:c1 - b0].bitcast(mmdt),
                                 start=False, stop=stop,
                                 skip_group_check=True)
                nc.tensor.matmul(psB[half][:, c0 - half * 512:c1 - half * 512],
                                 lhsT=vmat[:, jt, 2:5].bitcast(mmdt),
                                 rhs=w[:, c0 - b0:c1 - b0].bitcast(mmdt),
                                 start=False, stop=stop,
                                 skip_group_check=True)

        # ---- reshape sums: (5|3, 1024) -> S (128, g, 8) via PE transpose
        A8 = const.tile([5, NQ], f32)
        B8 = const.tile([3, NQ], f32)
        nc.vector.tensor_copy(A8[:, 0:512], psA[0])
        nc.vector.tensor_copy(A8[:, 512:NQ], psA[1])
        nc.scalar.copy(B8[:, 0:512], psB[0])
        nc.scalar.copy(B8[:, 512:NQ], psB[1])
        for g in range(G):
            psta = psumt.tile([P, 5], f32, tag="pst", name=f"psta{g}")
            nc.tensor.transpose(psta, A8[:, g * P:(g + 1) * P],
                                ident8[0:5, 0:5])
            nc.scalar.copy(S[:, g, 0:5], psta)
            pstb = psumt.tile([P, 3], f32, tag="pst", name=f"pstb{g}")
            nc.tensor.transpose(pstb, B8[:, g * P:(g + 1) * P],
                                ident8[0:3, 0:3])
            nc.vector.tensor_copy(S[:, g, 5:8], pstb)

        # ---- epilogue: xy-paired (128, G, 2) tiles where possible -----
        CNT = S[:, :, 4]
        uid = [0]

        def newt(dtype=f32, pair=False):
            uid[0] += 1
            shape = [P, G, 2] if pair else [P, G]
            return ep.tile(shape, dtype, tag=f"t{uid[0]}", name=f"t{uid[0]}")

        def tt2(a, b_, op):
            o = newt(pair=True)
            nc.vector.tensor_tensor(o, a, b_, op=op)
            return o

        def ts2(a, sc, op):
            o = newt(pair=True)
            nc.vector.tensor_scalar(o, a, float(sc), None, op0=op)
            return o

        def dup(a):
            o = newt(pair=True)
            nc.vector.tensor_copy(o[:, :, 0], a)
            nc.vector.tensor_copy(o[:, :, 1], a)
            return o

        # inv_cnt = 1 / (cnt + EPS); avg = sum * inv
        cnte = ep.tile([P, G], f32, tag="cnte")
        nc.vector.tensor_scalar(cnte, CNT, float(EPS), None, op0=Alu.add)
        inv = ep.tile([P, G], f32, tag="inv")
        nc.vector.reciprocal(inv, cnte)
        invP = dup(inv)
        wsP = dup(S[:, :, 7])

        al = tt2(tt2(S[:, :, 0:2], invP, Alu.mult), qv, Alu.subtract)
        co = tt2(tt2(S[:, :, 2:4], invP, Alu.mult), qp, Alu.subtract)
        sp = tt2(S[:, :, 5:7], tt2(qp, wsP, Alu.mult), Alu.subtract)
        # steer = (align + coh) + 1.5 * sep   (AW=CW=1 multiplies are exact)
        st = tt2(tt2(al, co, Alu.add), ts2(sp, SW, Alu.mult), Alu.add)

        def limit(vP, cap):
            sq = tt2(vP, vP, Alu.mult)
            n2 = ep.tile([P, G], f32, tag=f"n2{uid[0]}", name=f"n2{uid[0]}")
            nc.vector.tensor_tensor(n2, sq[:, :, 0], sq[:, :, 1], op=Alu.add)
            n = newt()
            nc.scalar.activation(n, n2, Act.Sqrt)
            g_ = newt(u32)
            nc.vector.tensor_scalar(g_, n, float(cap), None, op0=Alu.is_gt)
            den = newt()
            nc.vector.select(den, g_, n, ones_g)
            rden = newt()
            nc.vector.reciprocal(rden, den)
            sc = ts2(tt2(vP, dup(rden), Alu.mult), cap, Alu.mult)
            o = newt(pair=True)
            g2 = newt(u32, pair=True)
            nc.vector.tensor_copy(g2[:, :, 0], g_)
            nc.vector.tensor_copy(g2[:, :, 1], g_)
            nc.vector.select(o, g2, sc, vP)
            return o

        st = limit(st, MAX_FORCE)
        pv = ep.tile([P, G, 2], f32, tag="pv")
        vn = limit(tt2(qv, st, Alu.add), MAX_SPEED)
        nc.vector.tensor_copy(pv, vn)

        # This container's jax lowers (x % 1.0) as x - round(x); for
        # x = pos + vel in (-0.3, 1.3) that is exactly x - (x >= 0.5).
        po = ep.tile([P, G, 2], f32, tag="po")
        px = tt2(qp, vn, Alu.add)              # pos + vel_new (DT=1)
        ge = ts2(px, 0.5, Alu.is_ge)           # == 1.0 where >= 0.5
        nc.vector.tensor_tensor(po, px, ge, op=Alu.subtract)

        nc.sync.dma_start(out=opos[:, :, :], in_=po)
        nc.gpsimd.dma_start(out=ovel[:, :, :], in_=pv)

    nc.compile()
    return nc


def _get_nc(full=False):
    key = bool(full)
    if key not in _NC:
        _NC[key] = _build_nc(full=key)
    return _NC[key]


def _sort_inputs(pos, vel):
    """Sort each batch by x; return sorted arrays + permutations."""
    perms = []
    spos = np.empty_like(pos)
    svel = np.empty_like(vel)
    for b in range(B):
        perm = np.argsort(pos[b, :, 0], kind="stable")
        perms.append(perm)
        spos[b] = pos[b, perm]
        svel[b] = vel[b, perm]
    return spos, svel, perms


def _bands_ok(spos):
    """Exact check that the compiled bands cover every interacting
    (key-chunk, query) pair of this sorted input."""
    bands = _compute_bands(False)
    R = 0.1 + 1e-5
    for b in range(B):
        sx = spos[b, :, 0]
        for h in range(2):
            kx = np.roll(sx, -h * NQ)
            q = kx[:NQ]
            for jt in range(JT):
                ck = kx[jt * P:(jt + 1) * P]
                kmin, kmax = ck.min(), ck.max()
                d = np.maximum(0, np.maximum(kmin - q, q - kmax))
                dw = np.minimum(d, np.maximum(
                    0, np.maximum(kmin - (q + 1), (q + 1) - kmax)))
                dw = np.minimum(dw, np.maximum(
                    0, np.maximum(kmin - (q - 1), (q - 1) - kmax)))
                inter = np.nonzero(dw <= R)[0]
                if inter.size == 0:
                    continue
                if bands[jt] is None:
                    return False
                b0, b1 = bands[jt]
                if inter[0] < b0 or inter[-1] >= b1:
                    return False
    return True


def _pmaj(a):
    """(T*128, 2) row-major -> (128, T, 2) partition-major."""
    t = a.shape[0] // P
    return np.ascontiguousarray(a.reshape(t, P, 2).transpose(1, 0, 2))


def _shard(spos, svel):
    in_maps = []
    for k in range(NCORES):
        b, h = k // 2, k % 2
        rows = slice(h * NQ, (h + 1) * NQ)
        qp = np.ascontiguousarray(spos[b, rows])
        kp = np.roll(spos[b], -h * NQ, axis=0)
        kv = np.roll(svel[b], -h * NQ, axis=0)
        in_maps.append({
            "qpos": _pmaj(qp), "qvel": _pmaj(svel[b, rows]),
            "qposT": np.ascontiguousarray(qp.T),
            "kpos": _pmaj(kp), "kvel": _pmaj(kv),
        })
    return in_maps


def _run(pos, vel, **kwargs):
    from concourse.bass_utils import run_bass_kernel_spmd
    spos, svel, perms = _sort_inputs(pos, vel)
    full = FORCE_FULL or not _bands_ok(spos)
    nc = _get_nc(full=full)
    res = run_bass_kernel_spmd(nc, _shard(spos, svel), list(range(NCORES)),
                               **kwargs)
    out_pos = np.empty((B, N, 2), np.float32)
    out_vel = np.empty((B, N, 2), np.float32)
    for k in range(NCORES):
        b, h = k // 2, k % 2
        rows = slice(h * NQ, (h + 1) * NQ)
        out_pos[b, perms[b][rows]] = \
            res.results[k]["opos"].transpose(1, 0, 2).reshape(NQ, 2)
        out_vel[b, perms[b][rows]] = \
            res.results[k]["ovel"].transpose(1, 0, 2).reshape(NQ, 2)
    return out_pos, out_vel, res


def kernel(pos, vel, steps):
    pos = np.ascontiguousarray(np.asarray(pos, dtype=np.float32))
    vel = np.ascontiguousarray(np.asarray(vel, dtype=np.float32))
    for _ in range(int(steps)):
        pos, vel, _res = _run(pos, vel)
    return pos, vel


# revision 29
# speedup vs baseline: 1.0430x; 1.0430x over previous
"""Boids simulation step on 8 TRN2 NeuronCores (Bass/Tile).

Problem: B=4 independent sims of N=2048 boids in 2D. One step:
  - N x N pairwise: mask = (dist2 < R2) & (dist2 > 0); w = mask / (dist2 + EPS)
  - alignment / cohesion: (mask @ vel, mask @ pos, row-count) * 1/(cnt+eps)
  - separation: (w @ pos) - pos * rowsum(w)
  - steer = limit(align + coh + 1.5*sep, 0.5); vel' = limit(vel + steer, 0.3)
  - pos' = (pos + vel') % 1.0   (this jax lowers % as x - round(x), so the
    wrap is exactly x - (x >= 0.5) for x in (-0.3, 1.3))

Sharding: 8 cores = 4 batches x 2 halves. The host sorts each batch by x;
each core owns 1024 contiguous sorted query rows and gets the full 2048
keys ROLLED so its queries sit at key rows [0, 1024). The roll makes both
the i==j diagonal position and the x-band structure core-independent, so
one SPMD kernel serves all cores.

x-band culling: boids interact only within x-distance 0.1, so a 128-key
chunk only needs the ~(0.1 + chunk width) * N sorted-index band of query
columns (BAND_D slack, verified exactly on the host per call; falls back
to a full-band kernel variant if the input is too non-uniform). This
skips ~56% of the N^2 work including 4 of 16 chunks entirely.

Per core, for each of the 12 active key chunks (128 keys x W queries):
  dx2 = Square(ximat + (-xj))   [ScalarE: fused sub+square, bitwise == ref]
  dy2 = Square(yimat + (-yj))   [ScalarE]
  dist2 = dx2 + dy2             [VectorE, bitwise == ref]
  mask = dist2 < R2             [VectorE, exact 0/1; diag chunk zeroed]
  d2e = dist2 + EPS             [ScalarE Identity+bias]
  r = ~1/d2e                    [VectorE custom op, ~3e-6 rel]
  w = mask * r                  [VectorE]
  PE (V5 stationary so weight loads are ~free, tiles moving):
    psA(5,cols) += V5.T @ mask; psB(3,cols) += P3.T @ w  (PSUM accum)
The (8, 1024) sums are PE-transposed to (128, chunk, 8) planar form and a
paired-xy epilogue mirrors the reference ops bit-for-bit where it matters.
All DRAM I/O uses partition-major (128, chunk, 2) layouts so DMAs are
contiguous (the host does the cheap reshapes + unsort).
"""
import os
import numpy as np
from contextlib import ExitStack

B = 4
N = 2048
NQ = 1024          # query rows per core
NCORES = 8
P = 128
JT = N // P        # 16 key chunks
G = NQ // P        # 8 query sub-tiles
R2 = np.float32(0.1 ** 2)
EPS = np.float32(1e-6)
SW = np.float32(1.5)
MAX_SPEED = np.float32(0.3)
MAX_FORCE = np.float32(0.5)

_NC = {}

GP_TT = os.environ.get("BOIDS_GP_TT", "0") == "1"   # dist2 add on GPSIMD
D2E_ACT = os.environ.get("BOIDS_D2E", "act") == "act"
F32R = os.environ.get("BOIDS_F32R", "0") == "1"
BCAST_MM = os.environ.get("BOIDS_BCAST", "dma") == "mm"
SQRT_WARM = os.environ.get("BOIDS_SQRTWARM", "1") == "1"
# x-band culling: boids sorted by x only interact within sorted-index
# distance ~ (0.1 + chunk width) * N. D = slack in sorted indices.
BAND_D = int(os.environ.get("BOIDS_BAND_D", "250"))
FORCE_FULL = os.environ.get("BOIDS_FULL", "0") == "1"


def _compute_bands(full):
    """Per key-chunk query-column band [b0, b1) (128-aligned) or None."""
    if full:
        return [(0, NQ)] * JT
    bands = []
    for jt in range(JT):
        lo, hi = jt * P - BAND_D, jt * P + P + BAND_D
        iv = None
        for off in (-N, 0, N):
            a, b_ = max(lo + off, 0), min(hi + off, NQ)
            if a < b_:
                assert iv is None  # arc < 1024 cannot clip to two intervals
                iv = (a, b_)
        if iv is None:
            bands.append(None)
        else:
            bands.append(((iv[0] // P) * P, min(NQ, ((iv[1] + P - 1) // P) * P)))
    return bands


def _build_nc(full=False):
    import concourse.tile as tile
    from concourse import mybir, bacc

    f32 = mybir.dt.float32
    u32 = mybir.dt.uint32
    Alu = mybir.AluOpType
    Act = mybir.ActivationFunctionType

    bands = _compute_bands(full)
    nc = bacc.Bacc("TRN2", target_bir_lowering=False, debug=False)

    # all I/O in partition-major (128, chunk, xy) layout so DMAs are
    # contiguous per partition; the host does the cheap reshapes
    qpos = nc.dram_tensor("qpos", [P, G, 2], f32, kind="ExternalInput")
    qvel = nc.dram_tensor("qvel", [P, G, 2], f32, kind="ExternalInput")
    qposT = nc.dram_tensor("qposT", [2, NQ], f32, kind="ExternalInput")
    kpos = nc.dram_tensor("kpos", [P, JT, 2], f32, kind="ExternalInput")
    kvel = nc.dram_tensor("kvel", [P, JT, 2], f32, kind="ExternalInput")
    opos = nc.dram_tensor("opos", [P, G, 2], f32, kind="ExternalOutput")
    ovel = nc.dram_tensor("ovel", [P, G, 2], f32, kind="ExternalOutput")

    with tile.TileContext(nc) as tc, ExitStack() as ctx:
        const = ctx.enter_context(tc.tile_pool(name="const", bufs=1))
        WORK_BUFS = int(os.environ.get("BOIDS_BUFS", "3"))
        work = ctx.enter_context(tc.tile_pool(name="work", bufs=WORK_BUFS))
        ep = ctx.enter_context(tc.tile_pool(name="ep", bufs=1))
        psum = ctx.enter_context(tc.tile_pool(name="psum", bufs=1, space="PSUM"))
        psumt = ctx.enter_context(tc.tile_pool(name="psumt", bufs=2, space="PSUM"))
        psumb = ctx.enter_context(tc.tile_pool(name="psumb", bufs=2, space="PSUM"))

        # ---- setup -----------------------------------------------------
        # Query coords broadcast across partitions (partition-stride-0 DMA),
        # split in halves across queues; first so distance tiles start ASAP.
        ximat = const.tile([P, NQ], f32)
        yimat = const.tile([P, NQ], f32)
        if BCAST_MM:
            qTx = const.tile([1, NQ], f32)
            qTy = const.tile([1, NQ], f32)
            nc.sync.dma_start(out=qTx, in_=qposT[0:1, :])
            nc.sync.dma_start(out=qTy, in_=qposT[1:2, :])
            ones1 = const.tile([1, P], f32)
            nc.vector.memset(ones1, 1.0)
            for half in range(2):
                cs = slice(half * 512, (half + 1) * 512)
                for c, dstm in ((0, ximat), (1, yimat)):
                    bc = psumb.tile([P, 512], f32, tag="bc", name=f"bc{c}{half}")
                    src = qTx if c == 0 else qTy
                    nc.tensor.matmul(bc, lhsT=ones1, rhs=src[0:1, cs],
                                     start=True, stop=True)
                    (nc.vector.tensor_copy if c == 0 else nc.scalar.copy)(
                        dstm[:, cs], bc)
        else:
            nc.sync.dma_start(out=ximat[:, 0:512],
                              in_=qposT[0:1, 0:512].to_broadcast((P, 512)))
            nc.gpsimd.dma_start(out=yimat[:, 0:512],
                                in_=qposT[1:2, 0:512].to_broadcast((P, 512)))
            nc.scalar.dma_start(out=ximat[:, 512:NQ],
                                in_=qposT[0:1, 512:NQ].to_broadcast((P, 512)))
            nc.sync.dma_start(out=yimat[:, 512:NQ],
                              in_=qposT[1:2, 512:NQ].to_broadcast((P, 512)))

        vmat = const.tile([P, JT, 5], f32)
        nc.gpsimd.dma_start(out=vmat[:, :, 0:2], in_=kvel[:, :, :])
        nc.scalar.dma_start(out=vmat[:, :, 2:4], in_=kpos[:, :, :])
        nc.vector.memset(vmat[:, :, 4], 1.0)

        nkx = const.tile([P, JT], f32)
        nky = const.tile([P, JT], f32)
        nc.vector.tensor_scalar_mul(nkx, vmat[:, :, 2], -1.0)
        nc.vector.tensor_scalar_mul(nky, vmat[:, :, 3], -1.0)

        qp = const.tile([P, G, 2], f32)
        qv = const.tile([P, G, 2], f32)
        nc.sync.dma_start(out=qp, in_=qpos[:, :, :])
        nc.gpsimd.dma_start(out=qv, in_=qvel[:, :, :])

        ones_g = const.tile([P, G], f32)
        nc.vector.memset(ones_g, 1.0)

        # anti-identity (1 everywhere, 0 on diagonal) for the i==j zeroing
        antiI = const.tile([P, P], f32)
        nc.vector.memset(antiI, 1.0)
        nc.gpsimd.affine_select(
            antiI, antiI, pattern=[[1, P]],
            compare_op=Alu.not_equal, fill=0.0,
            base=0, channel_multiplier=-1)

        # 8x8 identity for PE-transpose of the (8, 1024) sums
        ident8 = const.tile([8, 8], f32)
        nc.vector.memset(ident8, 1.0)
        nc.gpsimd.affine_select(
            ident8, ident8, pattern=[[1, 8]],
            compare_op=Alu.is_equal, fill=0.0,
            base=0, channel_multiplier=-1)

        S = const.tile([P, G, 8], f32)  # per-query sums, planar
        mmdt = mybir.dt.float32r if F32R else f32

        eps_col = const.tile([P, 1], f32)
        nc.vector.memset(eps_col, float(EPS))
        if SQRT_WARM:
            sqrt_warm = const.tile([P, 1], f32)
            nc.scalar.activation(sqrt_warm, eps_col, Act.Sqrt)

        zeros512 = const.tile([P, 512], f32)
        nc.gpsimd.memset(zeros512, 0.0)

        # ---- main N^2 loop (16 x (128 keys x banded queries)) ---------
        psA = [psum.tile([5, 512], f32, tag=f"psA{i}", name=f"psA{i}")
               for i in range(2)]
        psB = [psum.tile([3, 512], f32, tag=f"psB{i}", name=f"psB{i}")
               for i in range(2)]
        # pre-clear accumulators (bands leave per-element first-writers
        # mixed); lhsT values are irrelevant since rhs == 0
        for pt, lh in ((psA[0], antiI[:, 0:5]), (psA[1], antiI[:, 0:5]),
                       (psB[0], antiI[:, 0:3]), (psB[1], antiI[:, 0:3])):
            nc.tensor.matmul(pt, lhsT=lh, rhs=zeros512, start=True,
                             stop=False, skip_group_check=True)
        # Chunk order: process the writers of query-half 1 first so its
        # accumulators complete mid-loop and the reduction (staging copies,
        # PE transposes) overlaps the remaining chunks' compute.
        order = [jt for jt in (7, 8, 9, 10, 3, 4, 5, 6, 0, 1, 2, 11, 12,
                               13, 14, 15)
                 if bands[jt] is not None]
        assert sorted(order) == [jt for jt in range(JT)
                                 if bands[jt] is not None]
        # last chunk (in processing order) writing each 512-column half
        lastw = [None, None]
        for jt in order:
            b0, b1 = bands[jt]
            for half in range(2):
                if b0 < (half + 1) * 512 and b1 > half * 512:
                    lastw[half] = jt

        A8 = const.tile([5, NQ], f32)
        B8 = const.tile([3, NQ], f32)

        def emit_half_reduction(half):
            cs = slice(half * 512, (half + 1) * 512)
            (nc.vector.tensor_copy if half else nc.scalar.copy)(
                A8[:, cs], psA[half])
            (nc.scalar.copy if half else nc.vector.tensor_copy)(
                B8[:, cs], psB[half])
            for g in range(half * 4, half * 4 + 4):
                psta = psumt.tile([P, 5], f32, tag="pst", name=f"psta{g}")
                nc.tensor.transpose(psta, A8[:, g * P:(g + 1) * P],
                                    ident8[0:5, 0:5])
                nc.scalar.copy(S[:, g, 0:5], psta)
                pstb = psumt.tile([P, 3], f32, tag="pst", name=f"pstb{g}")
                nc.tensor.transpose(pstb, B8[:, g * P:(g + 1) * P],
                                    ident8[0:3, 0:3])
                nc.vector.tensor_copy(S[:, g, 5:8], pstb)

        for jt in order:
            b0, b1 = bands[jt]
            W = b1 - b0
            dx2 = work.tile([P, W], f32, tag="dx2", name="dx2")
            nc.scalar.activation(dx2, ximat[:, b0:b1], Act.Square,
                                 bias=nkx[:, jt:jt + 1], scale=1.0)
            dy2 = work.tile([P, W], f32, tag="dy2", name="dy2")
            nc.scalar.activation(dy2, yimat[:, b0:b1], Act.Square,
                                 bias=nky[:, jt:jt + 1], scale=1.0)
            dist2 = work.tile([P, W], f32, tag="dist2", name="dist2")
            (nc.gpsimd if GP_TT else nc.vector).tensor_tensor(
                dist2, dx2, dy2, op=Alu.add)

            mask = work.tile([P, W], f32, tag="mask", name="mask")
            nc.vector.tensor_scalar(mask, dist2, float(R2), None,
                                    op0=Alu.is_lt)
            if jt < G:  # this chunk contains the i==j diagonal
                off = jt * P - b0
                assert 0 <= off and off + P <= W
                sub = mask[:, off:off + P]
                nc.vector.tensor_tensor(sub, sub, antiI, op=Alu.mult)

            d2e = work.tile([P, W], f32, tag="d2e", name="d2e")
            if D2E_ACT:
                nc.scalar.activation(d2e, dist2, Act.Identity,
                                     bias=eps_col[:, 0:1], scale=1.0)
            else:
                nc.vector.tensor_scalar(d2e, dist2, float(EPS), None,
                                        op0=Alu.add)
            r = work.tile([P, W], f32, tag="r", name="r")
            nc.vector.reciprocal_approx_fast(out=r, in_=d2e)
            w = work.tile([P, W], f32, tag="w", name="w")
            nc.vector.tensor_tensor(w, mask, r, op=Alu.mult)

            for half in range(2):
                c0, c1 = max(b0, half * 512), min(b1, (half + 1) * 512)
                if c0 >= c1:
                    continue
                stop = (lastw[half] == jt)
                nc.tensor.matmul(psA[half][:, c0 - half * 512:c1 - half * 512],
                                 lhsT=vmat[:, jt, :].bitcast(mmdt),
                                 rhs=mask[:, c0 - b0:c1 - b0].bitcast(mmdt),
                                 start=False, stop=stop,
                                 skip_group_check=True)
                nc.tensor.matmul(psB[half][:, c0 - half * 512:c1 - half * 512],
                                 lhsT=vmat[:, jt, 2:5].bitcast(mmdt),
                                 rhs=w[:, c0 - b0:c1 - b0].bitcast(mmdt),
                                 start=False, stop=stop,
                                 skip_group_check=True)
            if jt == lastw[1]:
                emit_half_reduction(1)
        emit_half_reduction(0)

        # ---- epilogue: xy-paired (128, G, 2) tiles where possible -----
        CNT = S[:, :, 4]
        uid = [0]

        def newt(dtype=f32, pair=False):
            uid[0] += 1
            shape = [P, G, 2] if pair else [P, G]
            return ep.tile(shape, dtype, tag=f"t{uid[0]}", name=f"t{uid[0]}")

        def tt2(a, b_, op):
            o = newt(pair=True)
            nc.vector.tensor_tensor(o, a, b_, op=op)
            return o

        def ts2(a, sc, op):
            o = newt(pair=True)
            nc.vector.tensor_scalar(o, a, float(sc), None, op0=op)
            return o

        def dup(a):
            o = newt(pair=True)
            nc.vector.tensor_copy(o[:, :, 0], a)
            nc.vector.tensor_copy(o[:, :, 1], a)
            return o

        # inv_cnt = 1 / (cnt + EPS); avg = sum * inv
        cnte = ep.tile([P, G], f32, tag="cnte")
        nc.vector.tensor_scalar(cnte, CNT, float(EPS), None, op0=Alu.add)
        inv = ep.tile([P, G], f32, tag="inv")
        nc.vector.reciprocal(inv, cnte)
        invP = dup(inv)
        wsP = dup(S[:, :, 7])

        al = tt2(tt2(S[:, :, 0:2], invP, Alu.mult), qv, Alu.subtract)
        co = tt2(tt2(S[:, :, 2:4], invP, Alu.mult), qp, Alu.subtract)
        sp = tt2(S[:, :, 5:7], tt2(qp, wsP, Alu.mult), Alu.subtract)
        # steer = (align + coh) + 1.5 * sep   (AW=CW=1 multiplies are exact)
        st = tt2(tt2(al, co, Alu.add), ts2(sp, SW, Alu.mult), Alu.add)

        def limit(vP, cap):
            sq = tt2(vP, vP, Alu.mult)
            n2 = ep.tile([P, G], f32, tag=f"n2{uid[0]}", name=f"n2{uid[0]}")
            nc.vector.tensor_tensor(n2, sq[:, :, 0], sq[:, :, 1], op=Alu.add)
            n = newt()
            nc.scalar.activation(n, n2, Act.Sqrt)
            g_ = newt(u32)
            nc.vector.tensor_scalar(g_, n, float(cap), None, op0=Alu.is_gt)
            den = newt()
            nc.vector.select(den, g_, n, ones_g)
            rden = newt()
            nc.vector.reciprocal(rden, den)
            sc = ts2(tt2(vP, dup(rden), Alu.mult), cap, Alu.mult)
            o = newt(pair=True)
            g2 = newt(u32, pair=True)
            nc.vector.tensor_copy(g2[:, :, 0], g_)
            nc.vector.tensor_copy(g2[:, :, 1], g_)
            nc.vector.select(o, g2, sc, vP)
            return o

        st = limit(st, MAX_FORCE)
        pv = ep.tile([P, G, 2], f32, tag="pv")
        vn = limit(tt2(qv, st, Alu.add), MAX_SPEED)
        nc.vector.tensor_copy(pv, vn)

        # This container's jax lowers (x % 1.0) as x - round(x); for
        # x = pos + vel in (-0.3, 1.3) that is exactly x - (x >= 0.5).
        po = ep.tile([P, G, 2], f32, tag="po")
        px = tt2(qp, vn, Alu.add)              # pos + vel_new (DT=1)
        ge = ts2(px, 0.5, Alu.is_ge)           # == 1.0 where >= 0.5
        nc.vector.tensor_tensor(po, px, ge, op=Alu.subtract)

        nc.sync.dma_start(out=opos[:, :, :], in_=po)
        nc.gpsimd.dma_start(out=ovel[:, :, :], in_=pv)

    nc.compile()
    return nc


def _get_nc(full=False):
    key = bool(full)
    if key not in _NC:
        _NC[key] = _build_nc(full=key)
    return _NC[key]


def _sort_inputs(pos, vel):
    """Sort each batch by x; return sorted arrays + permutations."""
    perms = []
    spos = np.empty_like(pos)
    svel = np.empty_like(vel)
    for b in range(B):
        perm = np.argsort(pos[b, :, 0], kind="stable")
        perms.append(perm)
        spos[b] = pos[b, perm]
        svel[b] = vel[b, perm]
    return spos, svel, perms


def _bands_ok(spos):
    """Exact check that the compiled bands cover every interacting
    (key-chunk, query) pair of this sorted input."""
    bands = _compute_bands(False)
    R = 0.1 + 1e-5
    for b in range(B):
        sx = spos[b, :, 0]
        for h in range(2):
            kx = np.roll(sx, -h * NQ)
            q = kx[:NQ]
            for jt in range(JT):
                ck = kx[jt * P:(jt + 1) * P]
                kmin, kmax = ck.min(), ck.max()
                d = np.maximum(0, np.maximum(kmin - q, q - kmax))
                dw = np.minimum(d, np.maximum(
                    0, np.maximum(kmin - (q + 1), (q + 1) - kmax)))
                dw = np.minimum(dw, np.maximum(
                    0, np.maximum(kmin - (q - 1), (q - 1) - kmax)))
                inter = np.nonzero(dw <= R)[0]
                if inter.size == 0:
                    continue
                if bands[jt] is None:
                    return False
                b0, b1 = bands[jt]
                if inter[0] < b0 or inter[-1] >= b1:
                    return False
    return True


def _pmaj(a):
    """(T*128, 2) row-major -> (128, T, 2) partition-major."""
    t = a.shape[0] // P
    return np.ascontiguousarray(a.reshape(t, P, 2).transpose(1, 0, 2))


def _shard(spos, svel):
    in_maps = []
    for k in range(NCORES):
        b, h = k // 2, k % 2
        rows = slice(h * NQ, (h + 1) * NQ)
        qp = np.ascontiguousarray(spos[b, rows])
        kp = np.roll(spos[b], -h * NQ, axis=0)
        kv = np.roll(svel[b], -h * NQ, axis=0)
        in_maps.append({
            "qpos": _pmaj(qp), "qvel": _pmaj(svel[b, rows]),
            "qposT": np.ascontiguousarray(qp.T),
            "kpos": _pmaj(kp), "kvel": _pmaj(kv),
        })
    return in_maps


def _run(pos, vel, **kwargs):
    from concourse.bass_utils import run_bass_kernel_spmd
    spos, svel, perms = _sort_inputs(pos, vel)
    full = FORCE_FULL or not _bands_ok(spos)
    nc = _get_nc(full=full)
    res = run_bass_kernel_spmd(nc, _shard(spos, svel), list(range(NCORES)),
                               **kwargs)
    out_pos = np.empty((B, N, 2), np.float32)
    out_vel = np.empty((B, N, 2), np.float32)
    for k in range(NCORES):
        b, h = k // 2, k % 2
        rows = slice(h * NQ, (h + 1) * NQ)
        out_pos[b, perms[b][rows]] = \
            res.results[k]["opos"].transpose(1, 0, 2).reshape(NQ, 2)
        out_vel[b, perms[b][rows]] = \
            res.results[k]["ovel"].transpose(1, 0, 2).reshape(NQ, 2)
    return out_pos, out_vel, res


def kernel(pos, vel, steps):
    pos = np.ascontiguousarray(np.asarray(pos, dtype=np.float32))
    vel = np.ascontiguousarray(np.asarray(vel, dtype=np.float32))
    for _ in range(int(steps)):
        pos, vel, _res = _run(pos, vel)
    return pos, vel
